# revision 11
# baseline (speedup 1.0000x reference)
"""Trainium2 Bass kernel for nn_Attention_4398046511861.

Bahdanau-style attention:
    proj_e = einsum('sbe,ae->sba', enc, w_ae) + b_ae
    proj_d = einsum('bd,ad->ba', dec, w_ad) + b_ad
    scores = einsum('sba,ba->sb', proj_e, proj_d)
    alphas = softmax(scores, axis=0)          # over sequence
    out    = einsum('sb,sbe->be', alphas, enc)

Key algebraic rewrite: scores[s,b] = enc[s,b,:] @ v_b + const_b where
v_b = w_ae^T @ proj_d[b] and const_b = b_ae . proj_d[b].  const_b is
uniform over s, so it cancels in the softmax and is dropped.  This
turns the dominant [S,B,E]x[A,E] projection into a per-batch matvec and
makes the kernel purely memory bound (one streaming read of enc).

Sharding: data-parallel over batch, B=32 -> 4 batches per core x 8 cores.
enc ships as fp16 (randn data, no range risk; 11-bit mantissa), host
pre-permuted so every enc DMA lands one contiguous 8KB run per partition.

Per-core device program (natural layout [s_partition, e_free]; the whole
16.8MB slice is SBUF-resident so enc is read from HBM exactly once):
  - prologue: proj_d and v_b rows on PE, v broadcast to all partitions
    via GPSIMD.
  - scores (the streaming bottleneck) is split across two engine paths
    to balance load:
      * AMR path: DVE affine_mul_reduce (fused mult+reduce, 1x rate)
      * ACT path: DVE batched tensor_mul (2x rate, fp16) + ScalarE
        Copy-activation with accum_out doing the row-sum
  - softmax: DVE reduce_max + GPSIMD partition_all_reduce(max),
    ACT Exp (bias=-max) with fused accum_out row-sum,
    GPSIMD partition_all_reduce(add), DVE reciprocal.
  - context: PE matmuls (alpha column stationary, enc tile moving),
    PSUM-accumulated over the 16 s-chunks; ACT scales by 1/L.
"""

import numpy as np

import concourse.bass as bass
import concourse.tile as tile
from concourse import bacc, mybir
from concourse import bass_isa
from concourse.bass_utils import run_bass_kernel_spmd

F32 = mybir.dt.float32

S, B, E, A, D = 2048, 32, 1024, 128, 1024
NCORES = 8
BLOC = B // NCORES          # 4 batches per core
SCH = 128                   # sequence positions per chunk (partition dim)
NSCH = S // SCH             # 16 s-chunks per batch
QCH = 4                     # s-chunks per DMA supertile
NQ = NSCH // QCH            # 4 supertiles per batch

ENC_DT = mybir.dt.float16
ENC_NP = np.float16

# Of the 16 supertiles, how many take the DVE-mult + ACT-accum path
# (the rest take the DVE affine_mul_reduce path).  Chosen to balance
# VectorE and ScalarE busy time (measured: AMR 1223ns/chunk, batched
# mult 2297ns/supertile, ACT copy+accum 1334ns/chunk).
ACT_PATH = 9


def _use_act_path(b, q):
    return (q % 2 == 0) or (b == 1 and q == 1)


def build_kernel(enc_dt=ENC_DT):
    nc = bacc.Bacc("TRN2", debug=False)

    enc = nc.dram_tensor(
        "enc", [BLOC, NQ, 128, QCH * E], enc_dt, kind="ExternalInput"
    ).ap()
    dec_t = nc.dram_tensor("dec_t", [128, D // 128, BLOC], enc_dt, kind="ExternalInput").ap()
    w_ad_t = nc.dram_tensor("w_ad_t", [128, (D // 128) * A], enc_dt, kind="ExternalInput").ap()
    w_ae_in = nc.dram_tensor("w_ae", [A, E], enc_dt, kind="ExternalInput").ap()
    b_ad_in = nc.dram_tensor("b_ad", [A, 1], F32, kind="ExternalInput").ap()
    out = nc.dram_tensor("out", [BLOC, E], F32, kind="ExternalOutput").ap()

    from contextlib import ExitStack

    with tile.TileContext(nc) as tc:
        with ExitStack() as ctx:
            singles = ctx.enter_context(tc.tile_pool(name="singles", bufs=1))
            encp = ctx.enter_context(tc.tile_pool(name="encp", bufs=BLOC * NQ))
            scr = ctx.enter_context(tc.tile_pool(name="scr", bufs=3))
            prodp = ctx.enter_context(tc.tile_pool(name="prodp", bufs=2))
            pps = ctx.enter_context(tc.tile_pool(name="pps", bufs=1, space="PSUM"))
            pctx = ctx.enter_context(tc.tile_pool(name="pctx", bufs=2, space="PSUM"))

            # ---- ACT exp-table preload (overlaps the initial DMA wait) ------
            warm = singles.tile([1, 1], F32, name="warm")
            nc.vector.memset(warm, 0.0)
            warmo = singles.tile([1, 1], F32, name="warmo")
            nc.scalar.activation(
                out=warmo, in_=warm, func=mybir.ActivationFunctionType.Exp,
                bias=0.0, scale=1.0,
            )

            # ---- weight / decoder loads (separate HWDGE queue: ScalarE) -----
            w_ad_sb_flat = singles.tile([128, (D // 128) * A], enc_dt)
            nc.sync.dma_start(out=w_ad_sb_flat, in_=w_ad_t)
            w_ad_sb = w_ad_sb_flat.rearrange("p (c a) -> p c a", c=D // 128)
            dec_sb = singles.tile([128, D // 128, BLOC], enc_dt)
            nc.sync.dma_start(out=dec_sb, in_=dec_t)
            b_ad_sb = singles.tile([A, 1], F32)
            nc.sync.dma_start(out=b_ad_sb, in_=b_ad_in)
            w_ae_sb = singles.tile([A, E], enc_dt)
            nc.sync.dma_start(out=w_ae_sb, in_=w_ae_in)

            # ---- ACT exp-table preload (overlaps the initial DMA wait) ------
            warm = singles.tile([1, 1], F32, name="warm")
            nc.vector.memset(warm, 0.0)
            warmo = singles.tile([1, 1], F32, name="warmo")
            nc.scalar.activation(
                out=warmo, in_=warm, func=mybir.ActivationFunctionType.Exp,
                bias=0.0, scale=1.0,
            )

            # ---- weight / decoder loads (separate HWDGE queue: ScalarE) -----
            w_ad_sb_flat = singles.tile([128, (D // 128) * A], enc_dt)
            nc.sync.dma_start(out=w_ad_sb_flat, in_=w_ad_t)
            w_ad_sb = w_ad_sb_flat.rearrange("p (c a) -> p c a", c=D // 128)
            dec_sb = singles.tile([128, D // 128, BLOC], enc_dt)
            nc.sync.dma_start(out=dec_sb, in_=dec_t)
            b_ad_sb = singles.tile([A, 1], F32)
            nc.sync.dma_start(out=b_ad_sb, in_=b_ad_in)
            w_ae_sb = singles.tile([A, E], enc_dt)
            nc.sync.dma_start(out=w_ae_sb, in_=w_ae_in)

            # ---- enc streaming loads (bulk stream on the Sync HWDGE queue)
            etile = {}
            for b in range(BLOC):
                for q in range(NQ):
                    et = encp.tile([128, QCH, E], enc_dt, tag="enc", name=f"enc{b}_{q}")
                    nc.sync.dma_start(
                        out=et, in_=enc[b, q].rearrange("p (c e) -> p c e", c=QCH)
                    )
                    etile[b, q] = et

            # ---- proj_d [A, BLOC] = w_ad @ dec^T + b_ad ---------------------
            projd_ps = pps.tile([A, BLOC], F32, tag="projd")
            nd = D // 128
            for c in range(nd):
                nc.tensor.matmul(
                    projd_ps,
                    w_ad_sb[:, c, :],
                    dec_sb[:, c, :],
                    start=(c == 0),
                    stop=(c == nd - 1),
                )
            projd_sb = singles.tile([A, BLOC], enc_dt)
            nc.vector.tensor_scalar_add(projd_sb, projd_ps, b_ad_sb)

            # ---- v_b rows and their partition-broadcast ---------------------
            v_rep = []
            for b in range(BLOC):
                vps = pps.tile([1, E], F32, tag="vps")
                for h in range(2):
                    nc.tensor.matmul(
                        vps[:, h * 512 : (h + 1) * 512],
                        projd_sb[:, b : b + 1],
                        w_ae_sb[:, h * 512 : (h + 1) * 512],
                        start=True,
                        stop=True,
                    )
                vrow = singles.tile([1, E], enc_dt, tag=f"vrow{b}", name=f"vrow{b}")
                nc.scalar.copy(out=vrow, in_=vps)
                vr = singles.tile([128, E], enc_dt, tag=f"vrep{b}", name=f"vrep{b}")
                nc.gpsimd.partition_broadcast(vr, vrow, channels=128)
                v_rep.append(vr)

            # ---- main per-batch pipeline ------------------------------------
            for b in range(BLOC):
                # v_rep[b] broadcast over the supertile middle dim (step-0 AP)
                vr = v_rep[b]
                v_bcast = bass.AP(
                    tensor=vr.tensor,
                    offset=vr.offset,
                    ap=[vr.ap[0], [0, QCH], vr.ap[1]],
                )
                sc = scr.tile([128, NSCH], F32, tag="scores")
                for q in range(NQ):
                    et = etile[b, q]
                    if _use_act_path(b, q):
                        prod4 = prodp.tile([128, QCH, E], enc_dt, tag="prod4")
                        nc.vector.tensor_mul(prod4, et, v_bcast)
                        for c in range(QCH):
                            j = q * QCH + c
                            dump = prodp.tile([128, E], enc_dt, tag="dump")
                            nc.scalar.activation(
                                out=dump,
                                in_=prod4[:, c, :],
                                func=mybir.ActivationFunctionType.Copy,
                                bias=0.0,
                                scale=1.0,
                                accum_out=sc[:, j : j + 1],
                            )
                    else:
                        for c in range(QCH):
                            j = q * QCH + c
                            tout = prodp.tile([128, E], enc_dt, tag="amrout")
                            nc.vector.affine_mul_reduce(
                                tout,
                                sc[:, j : j + 1],
                                et[:, c, :],
                                vr,
                                scale=1.0,
                                bias=0.0,
                            )

                # softmax over all 2048 scores of this batch
                rmax = scr.tile([128, 1], F32, tag="rmax")
                nc.vector.reduce_max(out=rmax, in_=sc, axis=mybir.AxisListType.X)
                gmax = scr.tile([128, 1], F32, tag="gmax")
                nc.gpsimd.partition_all_reduce(gmax, rmax, 128, bass_isa.ReduceOp.max)
                negmax = scr.tile([128, 1], F32, tag="negmax")
                nc.vector.tensor_scalar_mul(negmax, gmax, -1.0)
                al = scr.tile([128, NSCH], enc_dt, tag="alpha")
                rowsum = scr.tile([128, 1], F32, tag="rowsum")
                nc.scalar.activation(
                    out=al,
                    in_=sc,
                    func=mybir.ActivationFunctionType.Exp,
                    bias=negmax,
                    scale=1.0,
                    accum_out=rowsum,
                )
                lsum = scr.tile([128, 1], F32, tag="lsum")
                nc.gpsimd.partition_all_reduce(lsum, rowsum, 128, bass_isa.ReduceOp.add)
                linv = scr.tile([128, 1], F32, tag="linv")
                nc.vector.reciprocal(linv, lsum)

                # context[e] = sum_s alpha[s] * enc[s, e], accumulated in PSUM
                cps = [
                    pctx.tile([1, 512], F32, tag=f"cps{h}", name=f"cps{h}")
                    for h in range(2)
                ]
                for q in range(NQ):
                    for c in range(QCH):
                        j = q * QCH + c
                        for h in range(2):
                            nc.tensor.matmul(
                                cps[h],
                                al[:, j : j + 1],
                                etile[b, q][:, c, h * 512 : (h + 1) * 512],
                                start=(j == 0),
                                stop=(j == NSCH - 1),
                            )

                if b == BLOC - 2:
                    # keep the PE HAM-warm through the b3 softmax gap so the
                    # tail context matmuls run at 2.4 GHz
                    wps = pctx.tile([1, 512], F32, tag="warm", name="warm", bufs=1)
                    for wq in range(8):
                        nc.tensor.matmul(
                            wps,
                            al[:, wq : wq + 1],
                            etile[b, 0][:, wq % QCH, 0:512],
                            start=True,
                            stop=True,
                        )

                ob = scr.tile([1, E], F32, tag="outrow")
                for h in range(2):
                    nc.scalar.activation(
                        out=ob[:, h * 512 : (h + 1) * 512],
                        in_=cps[h],
                        func=mybir.ActivationFunctionType.Copy,
                        bias=0.0,
                        scale=linv[0:1, :],
                    )
                    nc.scalar.dma_start(
                        out=out[b : b + 1, h * 512 : (h + 1) * 512],
                        in_=ob[:, h * 512 : (h + 1) * 512],
                    )

    nc.compile()
    return nc


_NC_CACHE = {}


def _get_nc():
    if "nc" not in _NC_CACHE:
        _NC_CACHE["nc"] = build_kernel()
    return _NC_CACHE["nc"]


def make_in_maps(enc_outputs, dec_output, w_ae, w_ad, b_ad):
    enc16 = np.asarray(enc_outputs, dtype=np.float32).astype(ENC_NP)
    dec = np.asarray(dec_output, dtype=np.float32)
    # [A, D] -> [p, c, a] with d = c*128 + p (contiguous per-partition runs)
    w_ad_t = np.ascontiguousarray(
        np.asarray(w_ad, dtype=np.float32).T.reshape(D // 128, 128, A)
        .transpose(1, 0, 2).reshape(128, (D // 128) * A)
    ).astype(ENC_NP)
    w_ae_c = np.ascontiguousarray(np.asarray(w_ae, dtype=np.float32)).astype(ENC_NP)
    b_ad_c = np.asarray(b_ad, dtype=np.float32).reshape(A, 1)
    # [S, B, E] -> per-core [b, q, p, c, e] with s = q*512 + c*128 + p, so each
    # (b, q) DMA reads one contiguous 8KB run per partition.
    encp = enc16.reshape(NQ, QCH, 128, B, E).transpose(3, 0, 2, 1, 4)
    in_maps = []
    for core in range(NCORES):
        b0 = core * BLOC
        in_maps.append(
            {
                "enc": np.ascontiguousarray(
                    encp[b0 : b0 + BLOC].reshape(BLOC, NQ, 128, QCH * E)
                ),
                "dec_t": np.ascontiguousarray(
                    dec[b0 : b0 + BLOC, :].T.reshape(D // 128, 128, BLOC)
                    .transpose(1, 0, 2)
                ).astype(ENC_NP),
                "w_ad_t": w_ad_t,
                "w_ae": w_ae_c,
                "b_ad": b_ad_c,
            }
        )
    return in_maps


def kernel(enc_outputs, dec_output, w_ae, b_ae, w_ad, b_ad, _trace=False):
    """Full-input / full-output entry point.  b_ae is algebraically inert
    (uniform shift over the softmax axis) and is ignored."""
    nc = _get_nc()
    in_maps = make_in_maps(enc_outputs, dec_output, w_ae, w_ad, b_ad)
    res = run_bass_kernel_spmd(nc, in_maps, core_ids=list(range(NCORES)), trace=_trace)
    out = np.concatenate([r["out"] for r in res.results], axis=0)
    if _trace:
        return out, res
    return out


# revision 12
# speedup vs baseline: 1.0032x; 1.0032x over previous
"""Trainium2 Bass kernel for nn_Attention_4398046511861.

Bahdanau-style attention:
    proj_e = einsum('sbe,ae->sba', enc, w_ae) + b_ae
    proj_d = einsum('bd,ad->ba', dec, w_ad) + b_ad
    scores = einsum('sba,ba->sb', proj_e, proj_d)
    alphas = softmax(scores, axis=0)          # over sequence
    out    = einsum('sb,sbe->be', alphas, enc)

Key algebraic rewrite: scores[s,b] = enc[s,b,:] @ v_b + const_b where
v_b = w_ae^T @ proj_d[b] and const_b = b_ae . proj_d[b].  const_b is
uniform over s, so it cancels in the softmax and is dropped.  This
turns the dominant [S,B,E]x[A,E] projection into a per-batch matvec and
makes the kernel purely memory bound (one streaming read of enc).

Sharding: data-parallel over batch, B=32 -> 4 batches per core x 8 cores.
enc ships as fp16 (randn data, no range risk; 11-bit mantissa), host
pre-permuted so every enc DMA lands one contiguous 8KB run per partition.

Per-core device program (natural layout [s_partition, e_free]; the whole
16.8MB slice is SBUF-resident so enc is read from HBM exactly once):
  - prologue: proj_d and v_b rows on PE, v broadcast to all partitions
    via GPSIMD.
  - scores (the streaming bottleneck) is split across two engine paths
    to balance load:
      * AMR path: DVE affine_mul_reduce (fused mult+reduce, 1x rate)
      * ACT path: DVE batched tensor_mul (2x rate, fp16) + ScalarE
        Copy-activation with accum_out doing the row-sum
  - softmax: DVE reduce_max + GPSIMD partition_all_reduce(max),
    ACT Exp (bias=-max) with fused accum_out row-sum,
    GPSIMD partition_all_reduce(add), DVE reciprocal.
  - context: PE matmuls (alpha column stationary, enc tile moving),
    PSUM-accumulated over the 16 s-chunks; ACT scales by 1/L.
"""

import numpy as np

import concourse.bass as bass
import concourse.tile as tile
from concourse import bacc, mybir
from concourse import bass_isa
from concourse.bass_utils import run_bass_kernel_spmd

F32 = mybir.dt.float32

S, B, E, A, D = 2048, 32, 1024, 128, 1024
NCORES = 8
BLOC = B // NCORES          # 4 batches per core
SCH = 128                   # sequence positions per chunk (partition dim)
NSCH = S // SCH             # 16 s-chunks per batch
QCH = 4                     # s-chunks per DMA supertile
NQ = NSCH // QCH            # 4 supertiles per batch

ENC_DT = mybir.dt.float16
ENC_NP = np.float16

# Of the 16 supertiles, how many take the DVE-mult + ACT-accum path
# (the rest take the DVE affine_mul_reduce path).  Chosen to balance
# VectorE and ScalarE busy time (measured: AMR 1223ns/chunk, batched
# mult 2297ns/supertile, ACT copy+accum 1334ns/chunk).
ACT_PATH = 8


def _use_act_path(b, q):
    return q % 2 == 0


def build_kernel(enc_dt=ENC_DT):
    nc = bacc.Bacc("TRN2", debug=False)

    enc = nc.dram_tensor(
        "enc", [BLOC, NQ, 128, QCH * E], enc_dt, kind="ExternalInput"
    ).ap()
    dec_t = nc.dram_tensor("dec_t", [128, D // 128, BLOC], enc_dt, kind="ExternalInput").ap()
    w_ad_t = nc.dram_tensor("w_ad_t", [128, (D // 128) * A], enc_dt, kind="ExternalInput").ap()
    w_ae_in = nc.dram_tensor("w_ae", [A, E], enc_dt, kind="ExternalInput").ap()
    b_ad_in = nc.dram_tensor("b_ad", [A, 1], F32, kind="ExternalInput").ap()
    out = nc.dram_tensor("out", [BLOC, E], F32, kind="ExternalOutput").ap()

    from contextlib import ExitStack

    with tile.TileContext(nc) as tc:
        with ExitStack() as ctx:
            singles = ctx.enter_context(tc.tile_pool(name="singles", bufs=1))
            encp = ctx.enter_context(tc.tile_pool(name="encp", bufs=BLOC * NQ))
            scr = ctx.enter_context(tc.tile_pool(name="scr", bufs=3))
            prodp = ctx.enter_context(tc.tile_pool(name="prodp", bufs=2))
            pps = ctx.enter_context(tc.tile_pool(name="pps", bufs=1, space="PSUM"))
            pctx = ctx.enter_context(tc.tile_pool(name="pctx", bufs=2, space="PSUM"))

            # ---- ACT exp-table preload (overlaps the initial DMA wait) ------
            warm = singles.tile([1, 1], F32, name="warm")
            nc.vector.memset(warm, 0.0)
            warmo = singles.tile([1, 1], F32, name="warmo")
            nc.scalar.activation(
                out=warmo, in_=warm, func=mybir.ActivationFunctionType.Exp,
                bias=0.0, scale=1.0,
            )

            # ---- weight / decoder loads (separate HWDGE queue: ScalarE) -----
            w_ad_sb_flat = singles.tile([128, (D // 128) * A], enc_dt)
            nc.sync.dma_start(out=w_ad_sb_flat, in_=w_ad_t)
            w_ad_sb = w_ad_sb_flat.rearrange("p (c a) -> p c a", c=D // 128)
            dec_sb = singles.tile([128, D // 128, BLOC], enc_dt)
            nc.sync.dma_start(out=dec_sb, in_=dec_t)
            b_ad_sb = singles.tile([A, 1], F32)
            nc.sync.dma_start(out=b_ad_sb, in_=b_ad_in)
            w_ae_sb = singles.tile([A, E], enc_dt)
            nc.sync.dma_start(out=w_ae_sb, in_=w_ae_in)

            # ---- ACT exp-table preload (overlaps the initial DMA wait) ------
            warm = singles.tile([1, 1], F32, name="warm")
            nc.vector.memset(warm, 0.0)
            warmo = singles.tile([1, 1], F32, name="warmo")
            nc.scalar.activation(
                out=warmo, in_=warm, func=mybir.ActivationFunctionType.Exp,
                bias=0.0, scale=1.0,
            )

            # ---- weight / decoder loads (separate HWDGE queue: ScalarE) -----
            w_ad_sb_flat = singles.tile([128, (D // 128) * A], enc_dt)
            nc.sync.dma_start(out=w_ad_sb_flat, in_=w_ad_t)
            w_ad_sb = w_ad_sb_flat.rearrange("p (c a) -> p c a", c=D // 128)
            dec_sb = singles.tile([128, D // 128, BLOC], enc_dt)
            nc.sync.dma_start(out=dec_sb, in_=dec_t)
            b_ad_sb = singles.tile([A, 1], F32)
            nc.sync.dma_start(out=b_ad_sb, in_=b_ad_in)
            w_ae_sb = singles.tile([A, E], enc_dt)
            nc.sync.dma_start(out=w_ae_sb, in_=w_ae_in)

            # ---- enc streaming loads (bulk stream on the Sync HWDGE queue)
            etile = {}
            for b in range(BLOC):
                for q in range(NQ):
                    et = encp.tile([128, QCH, E], enc_dt, tag="enc", name=f"enc{b}_{q}")
                    nc.sync.dma_start(
                        out=et, in_=enc[b, q].rearrange("p (c e) -> p c e", c=QCH)
                    )
                    etile[b, q] = et

            # ---- proj_d [A, BLOC] = w_ad @ dec^T + b_ad ---------------------
            projd_ps = pps.tile([A, BLOC], F32, tag="projd")
            nd = D // 128
            for c in range(nd):
                nc.tensor.matmul(
                    projd_ps,
                    w_ad_sb[:, c, :],
                    dec_sb[:, c, :],
                    start=(c == 0),
                    stop=(c == nd - 1),
                )
            projd_sb = singles.tile([A, BLOC], enc_dt)
            nc.vector.tensor_scalar_add(projd_sb, projd_ps, b_ad_sb)

            # ---- v_b rows and their partition-broadcast ---------------------
            v_rep = []
            for b in range(BLOC):
                vps = pps.tile([1, E], F32, tag="vps")
                for h in range(2):
                    nc.tensor.matmul(
                        vps[:, h * 512 : (h + 1) * 512],
                        projd_sb[:, b : b + 1],
                        w_ae_sb[:, h * 512 : (h + 1) * 512],
                        start=True,
                        stop=True,
                    )
                vrow = singles.tile([1, E], enc_dt, tag=f"vrow{b}", name=f"vrow{b}")
                nc.scalar.copy(out=vrow, in_=vps)
                vr = singles.tile([128, E], enc_dt, tag=f"vrep{b}", name=f"vrep{b}")
                nc.gpsimd.partition_broadcast(vr, vrow, channels=128)
                v_rep.append(vr)

            # ---- main per-batch pipeline ------------------------------------
            for b in range(BLOC):
                # v_rep[b] broadcast over the supertile middle dim (step-0 AP)
                vr = v_rep[b]
                v_bcast = bass.AP(
                    tensor=vr.tensor,
                    offset=vr.offset,
                    ap=[vr.ap[0], [0, QCH], vr.ap[1]],
                )
                sc = scr.tile([128, NSCH], F32, tag="scores")
                score_insts = []
                for q in range(NQ):
                    et = etile[b, q]
                    if _use_act_path(b, q):
                        prod4 = prodp.tile([128, QCH, E], enc_dt, tag="prod4")
                        nc.vector.tensor_mul(prod4, et, v_bcast)
                        for c in range(QCH):
                            j = q * QCH + c
                            dump = prodp.tile([128, E], enc_dt, tag="dump")
                            score_insts.append(
                                nc.scalar.activation(
                                    out=dump,
                                    in_=prod4[:, c, :],
                                    func=mybir.ActivationFunctionType.Copy,
                                    bias=0.0,
                                    scale=1.0,
                                    accum_out=sc[:, j : j + 1],
                                )
                            )
                    else:
                        for c in range(QCH):
                            j = q * QCH + c
                            tout = prodp.tile([128, E], enc_dt, tag="amrout")
                            score_insts.append(
                                nc.vector.affine_mul_reduce(
                                    tout,
                                    sc[:, j : j + 1],
                                    et[:, c, :],
                                    vr,
                                    scale=1.0,
                                    bias=0.0,
                                )
                            )

                if b == BLOC - 1 and BLOC >= 2:
                    # Paced PE filler matmuls: each waits on one of this
                    # batch's score chunks, spreading ~300ns of PE activity
                    # across the last scores phase so HAM never sees an idle
                    # MID window and the tail context matmuls run at 2.4 GHz.
                    from concourse.tile import add_dep_helper

                    wps = pctx.tile([1, 512], F32, tag="warm", name="warm", bufs=1)
                    pal = prev_al
                    for wi in range(8):
                        mm = nc.tensor.matmul(
                            wps,
                            pal[:, wi : wi + 1],
                            etile[b - 1, 0][:, wi % QCH, 0:512],
                            start=True,
                            stop=True,
                        )
                        dep = score_insts[min(2 * wi + 1, len(score_insts) - 1)]
                        add_dep_helper(mm.ins, dep.ins, reason="PE warm pacing")

                # softmax over all 2048 scores of this batch
                rmax = scr.tile([128, 1], F32, tag="rmax")
                nc.vector.reduce_max(out=rmax, in_=sc, axis=mybir.AxisListType.X)
                gmax = scr.tile([128, 1], F32, tag="gmax")
                nc.gpsimd.partition_all_reduce(gmax, rmax, 128, bass_isa.ReduceOp.max)
                negmax = scr.tile([128, 1], F32, tag="negmax")
                nc.vector.tensor_scalar_mul(negmax, gmax, -1.0)
                al = scr.tile([128, NSCH], enc_dt, tag="alpha")
                rowsum = scr.tile([128, 1], F32, tag="rowsum")
                nc.scalar.activation(
                    out=al,
                    in_=sc,
                    func=mybir.ActivationFunctionType.Exp,
                    bias=negmax,
                    scale=1.0,
                    accum_out=rowsum,
                )
                lsum = scr.tile([128, 1], F32, tag="lsum")
                nc.gpsimd.partition_all_reduce(lsum, rowsum, 128, bass_isa.ReduceOp.add)
                linv = scr.tile([128, 1], F32, tag="linv")
                nc.vector.reciprocal(linv, lsum)

                # context[e] = sum_s alpha[s] * enc[s, e], accumulated in PSUM
                cps = [
                    pctx.tile([1, 512], F32, tag=f"cps{h}", name=f"cps{h}")
                    for h in range(2)
                ]
                for q in range(NQ):
                    for c in range(QCH):
                        j = q * QCH + c
                        for h in range(2):
                            nc.tensor.matmul(
                                cps[h],
                                al[:, j : j + 1],
                                etile[b, q][:, c, h * 512 : (h + 1) * 512],
                                start=(j == 0),
                                stop=(j == NSCH - 1),
                            )

                prev_al = al
                ob = scr.tile([1, E], F32, tag="outrow")
                for h in range(2):
                    nc.scalar.activation(
                        out=ob[:, h * 512 : (h + 1) * 512],
                        in_=cps[h],
                        func=mybir.ActivationFunctionType.Copy,
                        bias=0.0,
                        scale=linv[0:1, :],
                    )
                    nc.scalar.dma_start(
                        out=out[b : b + 1, h * 512 : (h + 1) * 512],
                        in_=ob[:, h * 512 : (h + 1) * 512],
                    )

    nc.compile()
    return nc


_NC_CACHE = {}


def _get_nc():
    if "nc" not in _NC_CACHE:
        _NC_CACHE["nc"] = build_kernel()
    return _NC_CACHE["nc"]


def make_in_maps(enc_outputs, dec_output, w_ae, w_ad, b_ad):
    enc16 = np.asarray(enc_outputs, dtype=np.float32).astype(ENC_NP)
    dec = np.asarray(dec_output, dtype=np.float32)
    # [A, D] -> [p, c, a] with d = c*128 + p (contiguous per-partition runs)
    w_ad_t = np.ascontiguousarray(
        np.asarray(w_ad, dtype=np.float32).T.reshape(D // 128, 128, A)
        .transpose(1, 0, 2).reshape(128, (D // 128) * A)
    ).astype(ENC_NP)
    w_ae_c = np.ascontiguousarray(np.asarray(w_ae, dtype=np.float32)).astype(ENC_NP)
    b_ad_c = np.asarray(b_ad, dtype=np.float32).reshape(A, 1)
    # [S, B, E] -> per-core [b, q, p, c, e] with s = q*512 + c*128 + p, so each
    # (b, q) DMA reads one contiguous 8KB run per partition.
    encp = enc16.reshape(NQ, QCH, 128, B, E).transpose(3, 0, 2, 1, 4)
    in_maps = []
    for core in range(NCORES):
        b0 = core * BLOC
        in_maps.append(
            {
                "enc": np.ascontiguousarray(
                    encp[b0 : b0 + BLOC].reshape(BLOC, NQ, 128, QCH * E)
                ),
                "dec_t": np.ascontiguousarray(
                    dec[b0 : b0 + BLOC, :].T.reshape(D // 128, 128, BLOC)
                    .transpose(1, 0, 2)
                ).astype(ENC_NP),
                "w_ad_t": w_ad_t,
                "w_ae": w_ae_c,
                "b_ad": b_ad_c,
            }
        )
    return in_maps


def kernel(enc_outputs, dec_output, w_ae, b_ae, w_ad, b_ad, _trace=False):
    """Full-input / full-output entry point.  b_ae is algebraically inert
    (uniform shift over the softmax axis) and is ignored."""
    nc = _get_nc()
    in_maps = make_in_maps(enc_outputs, dec_output, w_ae, w_ad, b_ad)
    res = run_bass_kernel_spmd(nc, in_maps, core_ids=list(range(NCORES)), trace=_trace)
    out = np.concatenate([r["out"] for r in res.results], axis=0)
    if _trace:
        return out, res
    return out


# revision 13
# speedup vs baseline: 1.0216x; 1.0183x over previous
"""Trainium2 Bass kernel for nn_Attention_4398046511861.

Bahdanau-style attention:
    proj_e = einsum('sbe,ae->sba', enc, w_ae) + b_ae
    proj_d = einsum('bd,ad->ba', dec, w_ad) + b_ad
    scores = einsum('sba,ba->sb', proj_e, proj_d)
    alphas = softmax(scores, axis=0)          # over sequence
    out    = einsum('sb,sbe->be', alphas, enc)

Key algebraic rewrite: scores[s,b] = enc[s,b,:] @ v_b + const_b where
v_b = w_ae^T @ proj_d[b] and const_b = b_ae . proj_d[b].  const_b is
uniform over s, so it cancels in the softmax and is dropped.  This
turns the dominant [S,B,E]x[A,E] projection into a per-batch matvec and
makes the kernel purely memory bound (one streaming read of enc).

Sharding: data-parallel over batch, B=32 -> 4 batches per core x 8 cores.
enc ships as fp16 (randn data, no range risk; 11-bit mantissa), host
pre-permuted so every enc DMA lands one contiguous 8KB run per partition.

Per-core device program (natural layout [s_partition, e_free]; the whole
16.8MB slice is SBUF-resident so enc is read from HBM exactly once):
  - prologue: proj_d and v_b rows on PE, v broadcast to all partitions
    via GPSIMD.
  - scores (the streaming bottleneck) is split across two engine paths
    to balance load:
      * AMR path: DVE affine_mul_reduce (fused mult+reduce, 1x rate)
      * ACT path: DVE batched tensor_mul (2x rate, fp16) + ScalarE
        Copy-activation with accum_out doing the row-sum
  - softmax: DVE reduce_max + GPSIMD partition_all_reduce(max),
    ACT Exp (bias=-max) with fused accum_out row-sum,
    GPSIMD partition_all_reduce(add), DVE reciprocal.
  - context: PE matmuls (alpha column stationary, enc tile moving),
    PSUM-accumulated over the 16 s-chunks; ACT scales by 1/L.
"""

import numpy as np

import concourse.bass as bass
import concourse.tile as tile
from concourse import bacc, mybir
from concourse import bass_isa
from concourse.bass_utils import run_bass_kernel_spmd

F32 = mybir.dt.float32

S, B, E, A, D = 2048, 32, 1024, 128, 1024
NCORES = 8
BLOC = B // NCORES          # 4 batches per core
SCH = 128                   # sequence positions per chunk (partition dim)
NSCH = S // SCH             # 16 s-chunks per batch
QCH = 4                     # s-chunks per DMA supertile
NQ = NSCH // QCH            # 4 supertiles per batch

ENC_DT = mybir.dt.float16
ENC_NP = np.float16

# Of the 16 supertiles, how many take the DVE-mult + ACT-accum path
# (the rest take the DVE affine_mul_reduce path).  Chosen to balance
# VectorE and ScalarE busy time (measured: AMR 1223ns/chunk, batched
# mult 2297ns/supertile, ACT copy+accum 1334ns/chunk).
ACT_PATH = 8


def _use_act_path(b, q):
    return q % 2 == 0


def build_kernel(enc_dt=ENC_DT):
    nc = bacc.Bacc("TRN2", debug=False)

    enc = nc.dram_tensor(
        "enc", [BLOC, NQ, 128, QCH * E], enc_dt, kind="ExternalInput"
    ).ap()
    dec_t = nc.dram_tensor("dec_t", [128, D // 128, BLOC], enc_dt, kind="ExternalInput").ap()
    w_ad_t = nc.dram_tensor("w_ad_t", [128, (D // 128) * A], enc_dt, kind="ExternalInput").ap()
    w_ae_in = nc.dram_tensor("w_ae", [A, E], enc_dt, kind="ExternalInput").ap()
    b_ad_in = nc.dram_tensor("b_ad", [A, 1], F32, kind="ExternalInput").ap()
    out = nc.dram_tensor("out", [BLOC, E], F32, kind="ExternalOutput").ap()

    from contextlib import ExitStack

    with tile.TileContext(nc) as tc:
        with ExitStack() as ctx:
            singles = ctx.enter_context(tc.tile_pool(name="singles", bufs=1))
            encp = ctx.enter_context(tc.tile_pool(name="encp", bufs=BLOC * NQ))
            scr = ctx.enter_context(tc.tile_pool(name="scr", bufs=3))
            prodp = ctx.enter_context(tc.tile_pool(name="prodp", bufs=2))
            pps = ctx.enter_context(tc.tile_pool(name="pps", bufs=1, space="PSUM"))
            pctx = ctx.enter_context(tc.tile_pool(name="pctx", bufs=2, space="PSUM"))

            # ---- ACT exp-table preload (overlaps the initial DMA wait) ------
            warm = singles.tile([1, 1], F32, name="warm")
            nc.vector.memset(warm, 0.0)
            warmo = singles.tile([1, 1], F32, name="warmo")
            nc.scalar.activation(
                out=warmo, in_=warm, func=mybir.ActivationFunctionType.Exp,
                bias=0.0, scale=1.0,
            )

            # ---- weight / decoder loads (separate HWDGE queue: ScalarE) -----
            w_ad_sb_flat = singles.tile([128, (D // 128) * A], enc_dt)
            nc.sync.dma_start(out=w_ad_sb_flat, in_=w_ad_t)
            w_ad_sb = w_ad_sb_flat.rearrange("p (c a) -> p c a", c=D // 128)
            dec_sb = singles.tile([128, D // 128, BLOC], enc_dt)
            nc.sync.dma_start(out=dec_sb, in_=dec_t)
            b_ad_sb = singles.tile([A, 1], F32)
            nc.sync.dma_start(out=b_ad_sb, in_=b_ad_in)
            w_ae_sb = singles.tile([A, E], enc_dt)
            nc.sync.dma_start(out=w_ae_sb, in_=w_ae_in)

            # ---- ACT exp-table preload (overlaps the initial DMA wait) ------
            warm = singles.tile([1, 1], F32, name="warm")
            nc.vector.memset(warm, 0.0)
            warmo = singles.tile([1, 1], F32, name="warmo")
            nc.scalar.activation(
                out=warmo, in_=warm, func=mybir.ActivationFunctionType.Exp,
                bias=0.0, scale=1.0,
            )

            # ---- weight / decoder loads (separate HWDGE queue: ScalarE) -----
            w_ad_sb_flat = singles.tile([128, (D // 128) * A], enc_dt)
            nc.sync.dma_start(out=w_ad_sb_flat, in_=w_ad_t)
            w_ad_sb = w_ad_sb_flat.rearrange("p (c a) -> p c a", c=D // 128)
            dec_sb = singles.tile([128, D // 128, BLOC], enc_dt)
            nc.sync.dma_start(out=dec_sb, in_=dec_t)
            b_ad_sb = singles.tile([A, 1], F32)
            nc.sync.dma_start(out=b_ad_sb, in_=b_ad_in)
            w_ae_sb = singles.tile([A, E], enc_dt)
            nc.sync.dma_start(out=w_ae_sb, in_=w_ae_in)

            # ---- enc streaming loads (bulk stream on the Sync HWDGE queue)
            etile = {}
            for b in range(BLOC):
                for q in range(NQ):
                    et = encp.tile([128, QCH, E], enc_dt, tag="enc", name=f"enc{b}_{q}")
                    nc.sync.dma_start(
                        out=et, in_=enc[b, q].rearrange("p (c e) -> p c e", c=QCH)
                    )
                    etile[b, q] = et

            # ---- proj_d [A, BLOC] = w_ad @ dec^T + b_ad ---------------------
            projd_ps = pps.tile([A, BLOC], F32, tag="projd")
            nd = D // 128
            for c in range(nd):
                nc.tensor.matmul(
                    projd_ps,
                    w_ad_sb[:, c, :],
                    dec_sb[:, c, :],
                    start=(c == 0),
                    stop=(c == nd - 1),
                )
            projd_sb = singles.tile([A, BLOC], enc_dt)
            nc.vector.tensor_scalar_add(projd_sb, projd_ps, b_ad_sb)

            # ---- v_b rows and their partition-broadcast ---------------------
            v_rep = []
            for b in range(BLOC):
                vps = pps.tile([1, E], F32, tag="vps")
                for h in range(2):
                    nc.tensor.matmul(
                        vps[:, h * 512 : (h + 1) * 512],
                        projd_sb[:, b : b + 1],
                        w_ae_sb[:, h * 512 : (h + 1) * 512],
                        start=True,
                        stop=True,
                    )
                vrow = singles.tile([1, E], enc_dt, tag=f"vrow{b}", name=f"vrow{b}")
                nc.scalar.copy(out=vrow, in_=vps)
                vr = singles.tile([128, E], enc_dt, tag=f"vrep{b}", name=f"vrep{b}")
                nc.gpsimd.partition_broadcast(vr, vrow, channels=128)
                v_rep.append(vr)

            # ---- main per-batch pipeline ------------------------------------
            for b in range(BLOC):
                # v_rep[b] broadcast over the supertile middle dim (step-0 AP)
                vr = v_rep[b]
                v_bcast = bass.AP(
                    tensor=vr.tensor,
                    offset=vr.offset,
                    ap=[vr.ap[0], [0, QCH], vr.ap[1]],
                )
                sc = scr.tile([128, NSCH], F32, tag="scores")
                score_insts = []
                for q in range(NQ):
                    et = etile[b, q]
                    if _use_act_path(b, q):
                        prod4 = prodp.tile([128, QCH, E], enc_dt, tag="prod4")
                        nc.vector.tensor_mul(prod4, et, v_bcast)
                        for c in range(QCH):
                            j = q * QCH + c
                            dump = prodp.tile([128, E], enc_dt, tag="dump")
                            score_insts.append(
                                nc.scalar.activation(
                                    out=dump,
                                    in_=prod4[:, c, :],
                                    func=mybir.ActivationFunctionType.Copy,
                                    bias=0.0,
                                    scale=1.0,
                                    accum_out=sc[:, j : j + 1],
                                )
                            )
                    else:
                        for c in range(QCH):
                            j = q * QCH + c
                            tout = prodp.tile([128, E], enc_dt, tag="amrout")
                            score_insts.append(
                                nc.vector.affine_mul_reduce(
                                    tout,
                                    sc[:, j : j + 1],
                                    et[:, c, :],
                                    vr,
                                    scale=1.0,
                                    bias=0.0,
                                )
                            )

                if b == BLOC - 1 and BLOC >= 2:
                    # Paced PE filler matmuls: each waits on one of this
                    # batch's score chunks, spreading ~300ns of PE activity
                    # across the last scores phase so HAM never sees an idle
                    # MID window and the tail context matmuls run at 2.4 GHz.
                    from concourse.tile import add_dep_helper

                    wps = pctx.tile([1, 512], F32, tag="warm", name="warm", bufs=1)
                    pal = prev_al
                    for wi in range(8):
                        mm = nc.tensor.matmul(
                            wps,
                            pal[:, wi : wi + 1],
                            etile[b - 1, 0][:, wi % QCH, 0:512],
                            start=True,
                            stop=True,
                        )
                        dep = score_insts[min(2 * wi + 1, len(score_insts) - 1)]
                        add_dep_helper(mm.ins, dep.ins, reason="PE warm pacing")

                # softmax over all 2048 scores of this batch
                rmax = scr.tile([128, 1], F32, tag="rmax")
                nc.vector.reduce_max(out=rmax, in_=sc, axis=mybir.AxisListType.X)
                gmax = scr.tile([128, 1], F32, tag="gmax")
                nc.gpsimd.partition_all_reduce(gmax, rmax, 128, bass_isa.ReduceOp.max)
                negmax = scr.tile([128, 1], F32, tag="negmax")
                nc.vector.tensor_scalar_mul(negmax, gmax, -1.0)
                al = scr.tile([128, NSCH], enc_dt, tag="alpha")
                rowsum = scr.tile([128, 1], F32, tag="rowsum")
                nc.scalar.activation(
                    out=al,
                    in_=sc,
                    func=mybir.ActivationFunctionType.Exp,
                    bias=negmax,
                    scale=1.0,
                    accum_out=rowsum,
                )
                lsum = scr.tile([128, 1], F32, tag="lsum")
                nc.gpsimd.partition_all_reduce(lsum, rowsum, 128, bass_isa.ReduceOp.add)
                linv = scr.tile([128, 1], F32, tag="linv")
                nc.vector.reciprocal(linv, lsum)

                # context[e] = sum_s alpha[s] * enc[s, e], accumulated in PSUM
                cps = [
                    pctx.tile([1, 512], F32, tag=f"cps{h}", name=f"cps{h}")
                    for h in range(2)
                ]
                for q in range(NQ):
                    for c in range(QCH):
                        j = q * QCH + c
                        for h in range(2):
                            nc.tensor.matmul(
                                cps[h],
                                al[:, j : j + 1],
                                etile[b, q][:, c, h * 512 : (h + 1) * 512],
                                start=(j == 0),
                                stop=(j == NSCH - 1),
                            )

                prev_al = al
                ob = scr.tile([1, E], F32, tag="outrow")
                for h in range(2):
                    if b >= BLOC - 2:
                        # DVE is idle at the tail; keep ScalarE free so the
                        # last batch's Exp isn't stuck behind these in FIFO
                        nc.vector.tensor_scalar_mul(
                            ob[:, h * 512 : (h + 1) * 512], cps[h], linv[0:1, :]
                        )
                    else:
                        nc.scalar.activation(
                            out=ob[:, h * 512 : (h + 1) * 512],
                            in_=cps[h],
                            func=mybir.ActivationFunctionType.Copy,
                            bias=0.0,
                            scale=linv[0:1, :],
                        )
                    nc.scalar.dma_start(
                        out=out[b : b + 1, h * 512 : (h + 1) * 512],
                        in_=ob[:, h * 512 : (h + 1) * 512],
                    )

    nc.compile()
    return nc


_NC_CACHE = {}


def _get_nc():
    if "nc" not in _NC_CACHE:
        _NC_CACHE["nc"] = build_kernel()
    return _NC_CACHE["nc"]


def make_in_maps(enc_outputs, dec_output, w_ae, w_ad, b_ad):
    enc16 = np.asarray(enc_outputs, dtype=np.float32).astype(ENC_NP)
    dec = np.asarray(dec_output, dtype=np.float32)
    # [A, D] -> [p, c, a] with d = c*128 + p (contiguous per-partition runs)
    w_ad_t = np.ascontiguousarray(
        np.asarray(w_ad, dtype=np.float32).T.reshape(D // 128, 128, A)
        .transpose(1, 0, 2).reshape(128, (D // 128) * A)
    ).astype(ENC_NP)
    w_ae_c = np.ascontiguousarray(np.asarray(w_ae, dtype=np.float32)).astype(ENC_NP)
    b_ad_c = np.asarray(b_ad, dtype=np.float32).reshape(A, 1)
    # [S, B, E] -> per-core [b, q, p, c, e] with s = q*512 + c*128 + p, so each
    # (b, q) DMA reads one contiguous 8KB run per partition.
    encp = enc16.reshape(NQ, QCH, 128, B, E).transpose(3, 0, 2, 1, 4)
    in_maps = []
    for core in range(NCORES):
        b0 = core * BLOC
        in_maps.append(
            {
                "enc": np.ascontiguousarray(
                    encp[b0 : b0 + BLOC].reshape(BLOC, NQ, 128, QCH * E)
                ),
                "dec_t": np.ascontiguousarray(
                    dec[b0 : b0 + BLOC, :].T.reshape(D // 128, 128, BLOC)
                    .transpose(1, 0, 2)
                ).astype(ENC_NP),
                "w_ad_t": w_ad_t,
                "w_ae": w_ae_c,
                "b_ad": b_ad_c,
            }
        )
    return in_maps


def kernel(enc_outputs, dec_output, w_ae, b_ae, w_ad, b_ad, _trace=False):
    """Full-input / full-output entry point.  b_ae is algebraically inert
    (uniform shift over the softmax axis) and is ignored."""
    nc = _get_nc()
    in_maps = make_in_maps(enc_outputs, dec_output, w_ae, w_ad, b_ad)
    res = run_bass_kernel_spmd(nc, in_maps, core_ids=list(range(NCORES)), trace=_trace)
    out = np.concatenate([r["out"] for r in res.results], axis=0)
    if _trace:
        return out, res
    return out


# revision 14
# speedup vs baseline: 1.0459x; 1.0238x over previous
"""Trainium2 Bass kernel for nn_Attention_4398046511861.

Bahdanau-style attention:
    proj_e = einsum('sbe,ae->sba', enc, w_ae) + b_ae
    proj_d = einsum('bd,ad->ba', dec, w_ad) + b_ad
    scores = einsum('sba,ba->sb', proj_e, proj_d)
    alphas = softmax(scores, axis=0)          # over sequence
    out    = einsum('sb,sbe->be', alphas, enc)

Key algebraic rewrite: scores[s,b] = enc[s,b,:] @ v_b + const_b where
v_b = w_ae^T @ proj_d[b] and const_b = b_ae . proj_d[b].  const_b is
uniform over s, so it cancels in the softmax and is dropped.  This
turns the dominant [S,B,E]x[A,E] projection into a per-batch matvec and
makes the kernel purely memory bound (one streaming read of enc).

Sharding: data-parallel over batch, B=32 -> 4 batches per core x 8 cores.
enc ships as fp16 (randn data, no range risk; 11-bit mantissa), host
pre-permuted so every enc DMA lands one contiguous 8KB run per partition.

Per-core device program (natural layout [s_partition, e_free]; the whole
16.8MB slice is SBUF-resident so enc is read from HBM exactly once):
  - prologue: proj_d and v_b rows on PE, v broadcast to all partitions
    via GPSIMD.
  - scores (the streaming bottleneck) is split across two engine paths
    to balance load:
      * AMR path: DVE affine_mul_reduce (fused mult+reduce, 1x rate)
      * ACT path: DVE batched tensor_mul (2x rate, fp16) + ScalarE
        Copy-activation with accum_out doing the row-sum
  - softmax: DVE reduce_max + GPSIMD partition_all_reduce(max),
    ACT Exp (bias=-max) with fused accum_out row-sum,
    GPSIMD partition_all_reduce(add), DVE reciprocal.
  - context: PE matmuls (alpha column stationary, enc tile moving),
    PSUM-accumulated over the 16 s-chunks; ACT scales by 1/L.
"""

import numpy as np

import concourse.bass as bass
import concourse.tile as tile
from concourse import bacc, mybir
from concourse import bass_isa
from concourse.bass_utils import run_bass_kernel_spmd

F32 = mybir.dt.float32

S, B, E, A, D = 2048, 32, 1024, 128, 1024
NCORES = 8
BLOC = B // NCORES          # 4 batches per core
SCH = 128                   # sequence positions per chunk (partition dim)
NSCH = S // SCH             # 16 s-chunks per batch
QCH = 4                     # s-chunks per DMA supertile
NQ = NSCH // QCH            # 4 supertiles per batch

ENC_DT = mybir.dt.float16
ENC_NP = np.float16

# Of the 16 supertiles, how many take the DVE-mult + ACT-accum path
# (the rest take the DVE affine_mul_reduce path).  Chosen to balance
# VectorE and ScalarE busy time (measured: AMR 1223ns/chunk, batched
# mult 2297ns/supertile, ACT copy+accum 1334ns/chunk).
ACT_PATH = 8


def _use_act_path(b, q):
    return q % 2 == 0


# individual chunks pulled out of the AMR path onto the mult+ACT-accum path
# (per-chunk, non-batched mult) to fine-tune the DVE/ACT balance
_ACT_SINGLE = {(0, 1, 3), (1, 1, 3), (1, 3, 3), (2, 3, 3)}


def build_kernel(enc_dt=ENC_DT):
    nc = bacc.Bacc("TRN2", debug=False)

    enc = nc.dram_tensor(
        "enc", [BLOC, NQ, 128, QCH * E], enc_dt, kind="ExternalInput"
    ).ap()
    dec_t = nc.dram_tensor("dec_t", [128, D // 128, BLOC], enc_dt, kind="ExternalInput").ap()
    w_ad_t = nc.dram_tensor("w_ad_t", [128, (D // 128) * A], enc_dt, kind="ExternalInput").ap()
    w_ae_in = nc.dram_tensor("w_ae", [A, E], enc_dt, kind="ExternalInput").ap()
    b_ad_in = nc.dram_tensor("b_ad", [A, 1], F32, kind="ExternalInput").ap()
    out = nc.dram_tensor("out", [BLOC, E], F32, kind="ExternalOutput").ap()

    from contextlib import ExitStack

    with tile.TileContext(nc) as tc:
        with ExitStack() as ctx:
            singles = ctx.enter_context(tc.tile_pool(name="singles", bufs=1))
            encp = ctx.enter_context(tc.tile_pool(name="encp", bufs=BLOC * NQ))
            scr = ctx.enter_context(tc.tile_pool(name="scr", bufs=3))
            prodp = ctx.enter_context(tc.tile_pool(name="prodp", bufs=2))
            pps = ctx.enter_context(tc.tile_pool(name="pps", bufs=1, space="PSUM"))
            pctx = ctx.enter_context(tc.tile_pool(name="pctx", bufs=2, space="PSUM"))

            # ---- ACT exp-table preload (overlaps the initial DMA wait) ------
            warm = singles.tile([1, 1], F32, name="warm")
            nc.vector.memset(warm, 0.0)
            warmo = singles.tile([1, 1], F32, name="warmo")
            nc.scalar.activation(
                out=warmo, in_=warm, func=mybir.ActivationFunctionType.Exp,
                bias=0.0, scale=1.0,
            )

            # ---- weight / decoder loads (separate HWDGE queue: ScalarE) -----
            w_ad_sb_flat = singles.tile([128, (D // 128) * A], enc_dt)
            half = (D // 128) * A // 2
            nc.sync.dma_start(out=w_ad_sb_flat[:, 0:half], in_=w_ad_t[:, 0:half])
            nc.sync.dma_start(out=w_ad_sb_flat[:, half:], in_=w_ad_t[:, half:])
            w_ad_sb = w_ad_sb_flat.rearrange("p (c a) -> p c a", c=D // 128)
            dec_sb = singles.tile([128, D // 128, BLOC], enc_dt)
            nc.sync.dma_start(out=dec_sb, in_=dec_t)
            b_ad_sb = singles.tile([A, 1], F32)
            nc.sync.dma_start(out=b_ad_sb, in_=b_ad_in)
            w_ae_sb = singles.tile([A, E], enc_dt)
            nc.sync.dma_start(out=w_ae_sb, in_=w_ae_in)

            # ---- ACT exp-table preload (overlaps the initial DMA wait) ------
            warm = singles.tile([1, 1], F32, name="warm")
            nc.vector.memset(warm, 0.0)
            warmo = singles.tile([1, 1], F32, name="warmo")
            nc.scalar.activation(
                out=warmo, in_=warm, func=mybir.ActivationFunctionType.Exp,
                bias=0.0, scale=1.0,
            )

            # ---- weight / decoder loads (separate HWDGE queue: ScalarE) -----
            w_ad_sb_flat = singles.tile([128, (D // 128) * A], enc_dt)
            half = (D // 128) * A // 2
            nc.sync.dma_start(out=w_ad_sb_flat[:, 0:half], in_=w_ad_t[:, 0:half])
            nc.sync.dma_start(out=w_ad_sb_flat[:, half:], in_=w_ad_t[:, half:])
            w_ad_sb = w_ad_sb_flat.rearrange("p (c a) -> p c a", c=D // 128)
            dec_sb = singles.tile([128, D // 128, BLOC], enc_dt)
            nc.sync.dma_start(out=dec_sb, in_=dec_t)
            b_ad_sb = singles.tile([A, 1], F32)
            nc.sync.dma_start(out=b_ad_sb, in_=b_ad_in)
            w_ae_sb = singles.tile([A, E], enc_dt)
            nc.sync.dma_start(out=w_ae_sb, in_=w_ae_in)

            # ---- enc streaming loads (bulk stream on the Sync HWDGE queue)
            etile = {}
            for b in range(BLOC):
                for q in range(NQ):
                    et = encp.tile([128, QCH, E], enc_dt, tag="enc", name=f"enc{b}_{q}")
                    nc.sync.dma_start(
                        out=et, in_=enc[b, q].rearrange("p (c e) -> p c e", c=QCH)
                    )
                    etile[b, q] = et

            # ---- proj_d [A, BLOC] = w_ad @ dec^T + b_ad ---------------------
            projd_ps = pps.tile([A, BLOC], F32, tag="projd")
            nd = D // 128
            for c in range(nd):
                nc.tensor.matmul(
                    projd_ps,
                    w_ad_sb[:, c, :],
                    dec_sb[:, c, :],
                    start=(c == 0),
                    stop=(c == nd - 1),
                )
            projd_sb = singles.tile([A, BLOC], enc_dt)
            nc.vector.tensor_scalar_add(projd_sb, projd_ps, b_ad_sb)

            # ---- v_b rows and their partition-broadcast ---------------------
            v_rep = []
            for b in range(BLOC):
                vps = pps.tile([1, E], F32, tag="vps")
                for h in range(2):
                    nc.tensor.matmul(
                        vps[:, h * 512 : (h + 1) * 512],
                        projd_sb[:, b : b + 1],
                        w_ae_sb[:, h * 512 : (h + 1) * 512],
                        start=True,
                        stop=True,
                    )
                vrow = singles.tile([1, E], enc_dt, tag=f"vrow{b}", name=f"vrow{b}")
                nc.scalar.copy(out=vrow, in_=vps)
                vr = singles.tile([128, E], enc_dt, tag=f"vrep{b}", name=f"vrep{b}")
                nc.gpsimd.partition_broadcast(vr, vrow, channels=128)
                v_rep.append(vr)

            # ---- main per-batch pipeline ------------------------------------
            for b in range(BLOC):
                # v_rep[b] broadcast over the supertile middle dim (step-0 AP)
                vr = v_rep[b]
                v_bcast = bass.AP(
                    tensor=vr.tensor,
                    offset=vr.offset,
                    ap=[vr.ap[0], [0, QCH], vr.ap[1]],
                )
                sc = scr.tile([128, NSCH], F32, tag="scores")
                score_insts = []
                for q in range(NQ):
                    et = etile[b, q]
                    if _use_act_path(b, q):
                        prod4 = prodp.tile([128, QCH, E], enc_dt, tag="prod4")
                        nc.vector.tensor_mul(prod4, et, v_bcast)
                        for c in range(QCH):
                            j = q * QCH + c
                            dump = prodp.tile([128, E], enc_dt, tag="dump")
                            score_insts.append(
                                nc.scalar.activation(
                                    out=dump,
                                    in_=prod4[:, c, :],
                                    func=mybir.ActivationFunctionType.Copy,
                                    bias=0.0,
                                    scale=1.0,
                                    accum_out=sc[:, j : j + 1],
                                )
                            )
                    else:
                        for c in range(QCH):
                            j = q * QCH + c
                            if (b, q, c) in _ACT_SINGLE:
                                p1 = prodp.tile([128, E], enc_dt, tag="p1")
                                nc.vector.tensor_mul(p1, et[:, c, :], vr)
                                dump = prodp.tile([128, E], enc_dt, tag="dump")
                                score_insts.append(
                                    nc.scalar.activation(
                                        out=dump,
                                        in_=p1,
                                        func=mybir.ActivationFunctionType.Copy,
                                        bias=0.0,
                                        scale=1.0,
                                        accum_out=sc[:, j : j + 1],
                                    )
                                )
                                continue
                            tout = prodp.tile([128, E], enc_dt, tag="amrout")
                            score_insts.append(
                                nc.vector.affine_mul_reduce(
                                    tout,
                                    sc[:, j : j + 1],
                                    et[:, c, :],
                                    vr,
                                    scale=1.0,
                                    bias=0.0,
                                )
                            )

                if b == BLOC - 1 and BLOC >= 2:
                    # Paced PE filler matmuls: each waits on one of this
                    # batch's score chunks, spreading ~300ns of PE activity
                    # across the last scores phase so HAM never sees an idle
                    # MID window and the tail context matmuls run at 2.4 GHz.
                    from concourse.tile import add_dep_helper

                    wps = pctx.tile([1, 512], F32, tag="warm", name="warm", bufs=1)
                    pal = prev_al
                    for wi in range(8):
                        mm = nc.tensor.matmul(
                            wps,
                            pal[:, wi : wi + 1],
                            etile[b - 1, 0][:, wi % QCH, 0:512],
                            start=True,
                            stop=True,
                        )
                        dep = score_insts[min(2 * wi + 1, len(score_insts) - 1)]
                        add_dep_helper(mm.ins, dep.ins, reason="PE warm pacing")

                # softmax over all 2048 scores of this batch
                rmax = scr.tile([128, 1], F32, tag="rmax")
                nc.vector.reduce_max(out=rmax, in_=sc, axis=mybir.AxisListType.X)
                gmax = scr.tile([128, 1], F32, tag="gmax")
                nc.gpsimd.partition_all_reduce(gmax, rmax, 128, bass_isa.ReduceOp.max)
                negmax = scr.tile([128, 1], F32, tag="negmax")
                nc.vector.tensor_scalar_mul(negmax, gmax, -1.0)
                al = scr.tile([128, NSCH], enc_dt, tag="alpha")
                rowsum = scr.tile([128, 1], F32, tag="rowsum")
                nc.scalar.activation(
                    out=al,
                    in_=sc,
                    func=mybir.ActivationFunctionType.Exp,
                    bias=negmax,
                    scale=1.0,
                    accum_out=rowsum,
                )
                lsum = scr.tile([128, 1], F32, tag="lsum")
                nc.gpsimd.partition_all_reduce(lsum, rowsum, 128, bass_isa.ReduceOp.add)
                linv = scr.tile([128, 1], F32, tag="linv")
                nc.vector.reciprocal(linv, lsum)

                # context[e] = sum_s alpha[s] * enc[s, e], accumulated in PSUM
                cps = [
                    pctx.tile([1, 512], F32, tag=f"cps{h}", name=f"cps{h}")
                    for h in range(2)
                ]
                for q in range(NQ):
                    for c in range(QCH):
                        j = q * QCH + c
                        for h in range(2):
                            nc.tensor.matmul(
                                cps[h],
                                al[:, j : j + 1],
                                etile[b, q][:, c, h * 512 : (h + 1) * 512],
                                start=(j == 0),
                                stop=(j == NSCH - 1),
                            )

                prev_al = al
                ob = scr.tile([1, E], F32, tag="outrow")
                for h in range(2):
                    if b >= BLOC - 2:
                        # DVE is idle at the tail; keep ScalarE free so the
                        # last batch's Exp isn't stuck behind these in FIFO
                        nc.vector.tensor_scalar_mul(
                            ob[:, h * 512 : (h + 1) * 512], cps[h], linv[0:1, :]
                        )
                    else:
                        nc.scalar.activation(
                            out=ob[:, h * 512 : (h + 1) * 512],
                            in_=cps[h],
                            func=mybir.ActivationFunctionType.Copy,
                            bias=0.0,
                            scale=linv[0:1, :],
                        )
                    nc.scalar.dma_start(
                        out=out[b : b + 1, h * 512 : (h + 1) * 512],
                        in_=ob[:, h * 512 : (h + 1) * 512],
                    )

    nc.compile()
    return nc


_NC_CACHE = {}


def _get_nc():
    if "nc" not in _NC_CACHE:
        _NC_CACHE["nc"] = build_kernel()
    return _NC_CACHE["nc"]


def make_in_maps(enc_outputs, dec_output, w_ae, w_ad, b_ad):
    enc16 = np.asarray(enc_outputs, dtype=np.float32).astype(ENC_NP)
    dec = np.asarray(dec_output, dtype=np.float32)
    # [A, D] -> [p, c, a] with d = c*128 + p (contiguous per-partition runs)
    w_ad_t = np.ascontiguousarray(
        np.asarray(w_ad, dtype=np.float32).T.reshape(D // 128, 128, A)
        .transpose(1, 0, 2).reshape(128, (D // 128) * A)
    ).astype(ENC_NP)
    w_ae_c = np.ascontiguousarray(np.asarray(w_ae, dtype=np.float32)).astype(ENC_NP)
    b_ad_c = np.asarray(b_ad, dtype=np.float32).reshape(A, 1)
    # [S, B, E] -> per-core [b, q, p, c, e] with s = q*512 + c*128 + p, so each
    # (b, q) DMA reads one contiguous 8KB run per partition.
    encp = enc16.reshape(NQ, QCH, 128, B, E).transpose(3, 0, 2, 1, 4)
    in_maps = []
    for core in range(NCORES):
        b0 = core * BLOC
        in_maps.append(
            {
                "enc": np.ascontiguousarray(
                    encp[b0 : b0 + BLOC].reshape(BLOC, NQ, 128, QCH * E)
                ),
                "dec_t": np.ascontiguousarray(
                    dec[b0 : b0 + BLOC, :].T.reshape(D // 128, 128, BLOC)
                    .transpose(1, 0, 2)
                ).astype(ENC_NP),
                "w_ad_t": w_ad_t,
                "w_ae": w_ae_c,
                "b_ad": b_ad_c,
            }
        )
    return in_maps


def kernel(enc_outputs, dec_output, w_ae, b_ae, w_ad, b_ad, _trace=False):
    """Full-input / full-output entry point.  b_ae is algebraically inert
    (uniform shift over the softmax axis) and is ignored."""
    nc = _get_nc()
    in_maps = make_in_maps(enc_outputs, dec_output, w_ae, w_ad, b_ad)
    res = run_bass_kernel_spmd(nc, in_maps, core_ids=list(range(NCORES)), trace=_trace)
    out = np.concatenate([r["out"] for r in res.results], axis=0)
    if _trace:
        return out, res
    return out
